# revision 8
# baseline (speedup 1.0000x reference)
"""Cross-attention kernel for Trainium2, 8 NeuronCores, data-parallel over batch.

Math (per batch b, one batch per core), using weight-product folding:
    A  = Wq @ Wk.T        (host, fp32)   [D, D]
    Bw = Wk @ Wv          (host, fp32)   [D, F]
    U^T = A^T @ q^T       (device, bf16) [D, Nq]     == (q @ A)^T
    scoresT = y @ U^T / sqrt(E)          [Nk, Nq]    ( == k_proj @ q_proj^T )
    v  = y @ Bw                          [Nk, F]     ( == (y@Wk) @ Wv )
    out = softmax-over-keys(scores) @ v  [Nq, F]

This removes the separate k-projection entirely (q@Wq, y@Wk, kp@Wv of the
naive form collapse into U and v): 12.9 GMAC/core instead of 15.0.

Precision plan: everything bf16 with fp32 PSUM accumulation, except the
scores matmul where 3 of the 4 contraction chunk-pairs run as fp8(e4m3)
DoubleRow matmuls (2x PE throughput); the remaining 2 chunks stay bf16.
Scales: U8 = e4m3(8*U), y8 = e4m3(16*y), bf16 part uses bf16(128*U)*y so
every PSUM contribution is 128*U*y; exp activation folds 1/(128*32).
Measured end-to-end rel err ~1.5e-2 (gate 2e-2); all-bf16 is ~3e-3.

Layouts are feature-major ([feat_part, chunk, token]) so all matmuls
contract along SBUF partitions with zero on-device transposes; the host
pre-transposes/casts q,y once (cheap numpy).

softmax denominator: 1-column ones-matmuls riding the out-matmul lhsT,
like the attention out-phase of the previous version; no max-subtraction
(scores are bounded, |s|<~3).
"""

import numpy as np
import ml_dtypes
from contextlib import ExitStack

import concourse.bass as bass
import concourse.tile as tile
from concourse import bacc, mybir
from concourse.bass_utils import run_bass_kernel_spmd

P = 128
F32 = mybir.dt.float32
BF16 = mybir.dt.bfloat16
F8E4 = mybir.dt.float8e4

NP_BF16 = ml_dtypes.bfloat16
NP_F8E4 = ml_dtypes.float8_e4m3

# Problem shapes (hardcoded per contract)
B = 8
NQ = 2048
NK = 2048
D = 1024   # in_q_dim == in_dim == hid_q == out_dim

SU = 8.0    # fp8 quantization scale for U
SY = 16.0   # fp8 quantization scale for y
NPAIR_F8 = 3  # of the 4 contraction chunk-pairs in the scores matmul, how
              # many run as fp8 DoubleRow (rest bf16). 0 => all-bf16 scores.


def build_program(nq=NQ, nk=NK, d=D, npair_f8=NPAIR_F8):
    nc = bacc.Bacc(trn_type="TRN2")

    DC = d // P            # feature chunks (8)
    MC = nk // P           # key chunks (16)
    NBLK = 512
    NB = nq // NBLK        # query blocks (4)
    NSUB = NBLK // P       # 128-query subblocks per block (4)
    FCH = 2                # 512-wide chunks of the value dim
    CF8 = 2 * npair_f8     # feature chunks handled in fp8
    # combined psum scale: fp8 part (SU*U)*(SY*y); bf16 part (SU*SY*U)*y
    PSCALE = SU * SY

    qT = nc.dram_tensor("qT", [d, nq], BF16, kind="ExternalInput").ap()
    yT = nc.dram_tensor("yT", [d, nk], BF16, kind="ExternalInput").ap()
    y8 = nc.dram_tensor("y8", [d, nk], F8E4, kind="ExternalInput").ap()
    Aw = nc.dram_tensor("Aw", [d, d], BF16, kind="ExternalInput").ap()
    Bw = nc.dram_tensor("Bw", [d, d], BF16, kind="ExternalInput").ap()
    out = nc.dram_tensor("out", [nq, d], F32, kind="ExternalOutput").ap()

    qT_v = qT.rearrange("(c p) n -> p c n", p=P)     # [P, DC, nq]
    yT_v = yT.rearrange("(c p) n -> p c n", p=P)
    y8_v = y8.rearrange("(c p) n -> p c n", p=P)
    Aw_v = Aw.rearrange("(c p) e -> p c e", p=P)     # [P, DC, d]
    Bw_v = Bw.rearrange("(c p) f -> p c f", p=P)
    out_v = out.rearrange("(b p) f -> b p f", p=P)   # [nq//P, P, d]

    with tile.TileContext(nc) as tc, ExitStack() as ctx:
        consts = ctx.enter_context(tc.tile_pool(name="consts", bufs=1))
        a_pool = ctx.enter_context(tc.tile_pool(name="aw", bufs=1))
        bw_pool = ctx.enter_context(tc.tile_pool(name="bw", bufs=1))
        ybf_pool = ctx.enter_context(tc.tile_pool(name="ybf", bufs=1))
        y8_pool = ctx.enter_context(tc.tile_pool(name="y8", bufs=1))
        v_pool = ctx.enter_context(tc.tile_pool(name="vproj", bufs=1))
        qstage = ctx.enter_context(tc.tile_pool(name="qstage", bufs=2))
        u8_pool = ctx.enter_context(tc.tile_pool(name="u8", bufs=2))
        ubf_pool = ctx.enter_context(tc.tile_pool(name="ubf", bufs=2))
        eT_pool = ctx.enter_context(tc.tile_pool(name="eT", bufs=2))
        out_pool = ctx.enter_context(tc.tile_pool(name="outsb", bufs=2))
        small = ctx.enter_context(tc.tile_pool(name="small", bufs=6))
        psum_a = ctx.enter_context(
            tc.tile_pool(name="psum_a", bufs=3, space="PSUM"))
        psum_o = ctx.enter_context(
            tc.tile_pool(name="psum_o", bufs=4, space="PSUM"))
        psum_den = ctx.enter_context(
            tc.tile_pool(name="psum_den", bufs=1, space="PSUM"))

        ones_bf = consts.tile([P, 1], BF16)
        nc.vector.memset(ones_bf, 1.0)
        zbias = consts.tile([P, 1], F32)
        nc.vector.memset(zbias, 0.0)

        A_sb = a_pool.tile([P, DC, d], BF16)
        Bw_sb = bw_pool.tile([P, DC, d], BF16)
        yT_sb = ybf_pool.tile([P, DC, nk], BF16)
        y8_sb = y8_pool.tile([P, max(1, CF8), nk], F8E4)
        v_sb = v_pool.tile([P, MC, d], BF16)

        # ---- DMA schedule -------------------------------------------------
        # Three trigger rings (SP / ACT / GPSIMD), ordered to match the phase
        # schedule U0 U1 S0 S1 V ... .  DMA trigger instructions cost
        # ~600-900ns of the ISSUING engine, so the scalar engine gets only a
        # short trigger burst (it must be free for u8-copy activations by
        # ~12us), and all qT traffic goes to the otherwise-idle gpsimd ring.
        # A goes in 128-wide e-slices alternating sync/scalar so U0's psum
        # groups are never DMA-paced; each slice feeds one U group.
        qt0 = qstage.tile([P, DC, NBLK], BF16, tag="qstage", name="qt0")
        for ei in range(DC):
            eng = nc.sync if ei % 2 == 0 else nc.scalar
            eng.dma_start(A_sb[:, :, ei * P:(ei + 1) * P],
                          Aw_v[:, :, ei * P:(ei + 1) * P])
            nc.gpsimd.dma_start(qt0[:, ei, :], qT_v[:, ei, 0:NBLK])
        qt1 = qstage.tile([P, DC, NBLK], BF16, tag="qstage", name="qt1")
        nc.gpsimd.dma_start(qt1, qT_v[:, :, NBLK:2 * NBLK])
        for c in range(CF8):
            nc.scalar.dma_start(y8_sb[:, c, :], y8_v[:, c, :])
        for c in range(CF8, DC):
            nc.sync.dma_start(yT_sb[:, c, :], yT_v[:, c, :])
        for c in range(DC):
            nc.sync.dma_start(Bw_sb[:, c, :], Bw_v[:, c, :])
        for c in range(CF8):
            nc.scalar.dma_start(yT_sb[:, c, :], yT_v[:, c, :])

        def u_phase(nb, qt):
            """U^T[e, n-block] -> u8 (fp8, x SU) and ubf (bf16, x SU*SY)."""
            u8 = u8_pool.tile([P, max(1, CF8), NBLK], F8E4, tag="u8")
            ubf = ubf_pool.tile([P, max(1, DC - CF8), NBLK], BF16, tag="ubf")
            for ei in range(DC):
                ps = psum_a.tile([P, 512], F32, tag="psa", name="psa")
                for di in range(DC):
                    nc.tensor.matmul(
                        ps,
                        lhsT=A_sb[:, di, ei * P:(ei + 1) * P],
                        rhs=qt[:, di, :],
                        start=(di == 0), stop=(di == DC - 1))
                if ei < CF8:
                    nc.scalar.activation(
                        u8[:, ei, :], ps,
                        mybir.ActivationFunctionType.Copy, scale=SU)
                else:
                    nc.scalar.activation(
                        ubf[:, ei - CF8, :], ps,
                        mybir.ActivationFunctionType.Copy, scale=PSCALE)
            return u8, ubf

        def s_phase(nb, u8, ubf):
            """eT[m, n-block] = exp(scoresT / (PSCALE * sqrt(d)))."""
            eT = eT_pool.tile([P, MC, NBLK], BF16, tag="eT")
            for mi in range(MC):
                ps = psum_a.tile([P, 512], F32, tag="psa", name="psa")
                for nh in range(2):
                    pshalf = ps[:, nh * 256:(nh + 1) * 256]
                    nmm = npair_f8 + (DC - CF8)
                    k = 0
                    for c in range(npair_f8):
                        nc.tensor.matmul(
                            pshalf,
                            lhsT=y8_sb[:, 2 * c:2 * c + 2,
                                       mi * P:(mi + 1) * P],
                            rhs=u8[:, 2 * c:2 * c + 2,
                                   nh * 256:(nh + 1) * 256],
                            start=(k == 0), stop=(k == nmm - 1),
                            perf_mode=mybir.MatmulPerfMode.DoubleRow)
                        k += 1
                    for c in range(CF8, DC):
                        nc.tensor.matmul(
                            pshalf,
                            lhsT=yT_sb[:, c, mi * P:(mi + 1) * P],
                            rhs=ubf[:, c - CF8, nh * 256:(nh + 1) * 256],
                            start=(k == 0), stop=(k == nmm - 1))
                        k += 1
                nc.scalar.activation(
                    eT[:, mi, :], ps,
                    mybir.ActivationFunctionType.Exp,
                    bias=zbias, scale=1.0 / (PSCALE * float(np.sqrt(d))))
            return eT

        def v_phase():
            """v[m, f] = y @ Bw, bf16."""
            for mi in range(MC):
                for j in range(FCH):
                    ps = psum_a.tile([P, 512], F32, tag="psa", name="psa")
                    for di in range(DC):
                        nc.tensor.matmul(
                            ps,
                            lhsT=yT_sb[:, di, mi * P:(mi + 1) * P],
                            rhs=Bw_sb[:, di, 512 * j:512 * (j + 1)],
                            start=(di == 0), stop=(di == DC - 1))
                    nc.vector.tensor_copy(v_sb[:, mi, 512 * j:512 * (j + 1)], ps)

        def o_phase(nb, eT):
            """out[n, f] = (eT.T @ v) / (eT.T @ 1)."""
            for ns in range(NSUB):
                pos = [psum_o.tile([P, 512], F32, tag="pso", name="pso")
                       for _ in range(FCH)]
                pden = psum_den.tile([P, 1], F32, tag="pden", name="pden")
                for mi in range(MC):
                    lhsT_e = eT[:, mi, ns * P:(ns + 1) * P]
                    for j in range(FCH):
                        nc.tensor.matmul(
                            pos[j], lhsT=lhsT_e,
                            rhs=v_sb[:, mi, 512 * j:512 * (j + 1)],
                            start=(mi == 0), stop=(mi == MC - 1))
                    nc.tensor.matmul(
                        pden, lhsT=lhsT_e, rhs=ones_bf,
                        start=(mi == 0), stop=(mi == MC - 1))
                rec = small.tile([P, 1], F32)
                nc.vector.reciprocal(rec, pden)
                ob = out_pool.tile([P, d], F32)
                for j in range(FCH):
                    nc.vector.tensor_scalar_mul(
                        ob[:, 512 * j:512 * (j + 1)], pos[j], rec)
                    nc.sync.dma_start(
                        out_v[nb * NSUB + ns][:, 512 * j:512 * (j + 1)],
                        ob[:, 512 * j:512 * (j + 1)])

        # ---- phase schedule ----------------------------------------------
        # U0 U1 S0 S1 V O0 U2 S2 O1 U3 S3 O2 O3: U0/U1 are DMA-light (3MB)
        # and buy time for the S-phase and V-phase inputs to land; O(k)
        # follows U(k+2) so psum->sbuf copies overlap O's matmuls.
        def load_q(nb):
            qt = qstage.tile([P, DC, NBLK], BF16, tag="qstage", name="qt")
            nc.gpsimd.dma_start(qt, qT_v[:, :, nb * NBLK:(nb + 1) * NBLK])
            return qt

        u0 = u_phase(0, qt0)
        u1 = u_phase(1, qt1)
        e0 = s_phase(0, *u0)
        e1 = s_phase(1, *u1)
        v_phase()
        o_phase(0, e0)
        qt2 = load_q(2)
        u2 = u_phase(2, qt2)
        e2 = s_phase(2, *u2)
        o_phase(1, e1)
        qt3 = load_q(3)
        u3 = u_phase(3, qt3)
        e3 = s_phase(3, *u3)
        o_phase(2, e2)
        o_phase(3, e3)

    nc.compile()
    return nc


_CACHE = {}


def _prep(q, y, Wq, Wk, Wv):
    q = np.asarray(q, dtype=np.float32)
    y = np.asarray(y, dtype=np.float32)
    Wq = np.asarray(Wq, dtype=np.float32)
    Wk = np.asarray(Wk, dtype=np.float32)
    Wv = np.asarray(Wv, dtype=np.float32)
    A = (Wq @ Wk.T).astype(NP_BF16)
    Bw = (Wk @ Wv).astype(NP_BF16)
    in_maps = []
    for b in range(B):
        yTb = np.ascontiguousarray(y[b].T)
        in_maps.append({
            "qT": np.ascontiguousarray(q[b].T).astype(NP_BF16),
            "yT": yTb.astype(NP_BF16),
            "y8": (yTb * np.float32(SY)).astype(NP_F8E4),
            "Aw": A, "Bw": Bw,
        })
    return in_maps


def kernel(q, y, Wq, Wk, Wv):
    if "nc" not in _CACHE:
        _CACHE["nc"] = build_program()
    nc = _CACHE["nc"]
    in_maps = _prep(q, y, Wq, Wk, Wv)
    res = run_bass_kernel_spmd(nc, in_maps, core_ids=list(range(B)))
    return np.stack([res.results[b]["out"] for b in range(B)], axis=0)
